# revision 8
# baseline (speedup 1.0000x reference)
"""Trainium2 kernel for nn_CA_23175643529789 (dense_cnn, memory regime).

The reference network is:
    y  = depthwise3x3(x, dw_k, depth_multiplier=3) + dw_b      # 1 -> 3 ch
    h  = BN_0(relu(y @ w0 + b0))                               # 3 -> 1 ch
    h  = BN_{i+1}(relu(h * ws[i] + bs[i]))   for i in 0..9     # 1 -> 1 ch
    out = x + h * wf + bf

Everything after the depthwise conv is scalar arithmetic per pixel, so the
whole network folds (exactly, by linearity) into ONE 3x3 conv followed by a
chain of 11 scalar relu-affine stages:  v_{i+1} = alpha_i * relu(v_i) + beta_i,
with out = x + v_11.

At kernel-call time we know the actual weight values, so we propagate the
achievable value interval through the chain.  A stage whose input interval is
entirely <= 0 zeroes every pixel, making the rest of the chain a constant:
out = x + C.  (With the shipped weights this provably happens at stage 2 for
*any* input x, because alpha_1 < 0 and beta_1 < 0.)  The device kernel is then
a pure memory-roofline pass: read x, add C, write out, sharded over 8 cores.

If the collapse does not hold for the supplied weights, we fall back to an
exact host computation (correct, just not accelerated).
"""

import sys

import numpy as np

_REPO = "/opt/trn_rl_repo"
if _REPO not in sys.path:
    sys.path.insert(0, _REPO)

BN_EPS = 1e-3
N_CORES = 8

_PROG_CACHE: dict = {}


# --------------------------------------------------------------------------
# Host-side algebraic folding
# --------------------------------------------------------------------------

def _fold(dw_k, dw_b, w0, b0, ws, bs, gamma, beta, mmean, mvar, wf, bf):
    """Fold network into (K3x3, zbias, alphas[11], betas[11]) in float64."""
    f8 = np.float64
    K = np.einsum("dtj,j->dt", dw_k[:, :, 0, :].astype(f8), w0[:, 0].astype(f8))
    zb = float(np.dot(dw_b.astype(f8), w0[:, 0].astype(f8)) + f8(b0[0]))
    s = gamma[:, 0].astype(f8) / np.sqrt(mvar[:, 0].astype(f8) + BN_EPS)
    t = beta[:, 0].astype(f8) - mmean[:, 0].astype(f8) * s
    alphas, betas = [], []
    for i in range(10):
        alphas.append(float(s[i] * f8(ws[i, 0, 0])))
        betas.append(float(t[i] * f8(ws[i, 0, 0]) + f8(bs[i, 0])))
    alphas.append(float(s[10] * f8(wf[0, 0])))
    betas.append(float(t[10] * f8(wf[0, 0]) + f8(bf[0])))
    return K, zb, alphas, betas


def _find_collapse(K, zb, alphas, betas, x_absmax):
    """Interval-propagate; return stage index where relu provably zeroes
    every pixel (with margin), or None."""
    zr = float(np.abs(K).sum() * x_absmax)
    vlo, vhi = zb - zr, zb + zr
    for i in range(11):
        if vhi <= -1e-4:  # relu_i kills everything, with margin
            return i
        ulo, uhi = max(vlo, 0.0), max(vhi, 0.0)
        lo2 = alphas[i] * ulo + betas[i]
        hi2 = alphas[i] * uhi + betas[i]
        vlo, vhi = min(lo2, hi2), max(lo2, hi2)
    return None


def _collapsed_const(collapse_at, ws, bs, gamma, beta, mmean, mvar, wf, bf):
    """Replicate the reference's float32 arithmetic from block `collapse_at`
    (whose relu output is exactly 0 at every pixel) to the end."""
    f4 = np.float32
    gamma = gamma.astype(f4)
    beta = beta.astype(f4)
    mmean = mmean.astype(f4)
    mvar = mvar.astype(f4)
    ws = ws.astype(f4)
    bs = bs.astype(f4)

    def bn(u, k):
        return (u - mmean[k, 0]) * (gamma[k, 0] / np.sqrt(mvar[k, 0] + f4(BN_EPS))) + beta[k, 0]

    h = bn(f4(0.0), collapse_at)
    for k in range(collapse_at + 1, 11):
        h = bn(np.maximum(h * ws[k - 1, 0, 0] + bs[k - 1, 0], f4(0.0)), k)
    return f4(h * f4(wf[0, 0]) + f4(bf[0]))


# --------------------------------------------------------------------------
# Exact host fallback (only used if the collapse does not hold)
# --------------------------------------------------------------------------

def _host_reference(x, dw_k, dw_b, w0, b0, ws, bs, gamma, beta, mmean, mvar, wf, bf):
    f4 = np.float32
    B, H, W, C = x.shape
    xp = np.pad(x[..., 0], ((0, 0), (1, 1), (1, 1))).astype(f4)
    y = np.zeros((B, H, W, 3), dtype=f4)
    for j in range(3):
        acc = np.zeros((B, H, W), dtype=f4)
        for d in range(3):
            for tt in range(3):
                acc += dw_k[d, tt, 0, j] * xp[:, d : d + H, tt : tt + W]
        y[..., j] = acc + dw_b[j]

    def bn(u, k):
        return (u - mmean[k, 0]) * (gamma[k, 0] / np.sqrt(mvar[k, 0] + f4(BN_EPS))) + beta[k, 0]

    h = bn(np.maximum(y @ w0.astype(f4) + b0.astype(f4), 0.0)[..., 0], 0)
    for i in range(10):
        h = bn(np.maximum(h * ws[i, 0, 0] + bs[i, 0], 0.0), i + 1)
    dx = h * wf[0, 0] + bf[0]
    return (x + dx[..., None]).astype(f4)


# --------------------------------------------------------------------------
# Device program: out = x + C, sharded over 8 cores
# --------------------------------------------------------------------------

P = 128          # SBUF partitions
F_PER_CORE = 16384   # fp32 elems per partition per core (2*1024*1024 / 128)
# 3 chunks -> 6 dma_starts -> 6 DMAHW sem lanes + 1 DVE sem = 7 sync waits on
# the kernel-tail Drain (the CoreV3 Drain slot holds at most 8).
CHUNKS = (6144, 6144, 4096)


def _build_const_add(c: float):
    import concourse.tile as tile
    from concourse import bacc, mybir

    # Bacc (not plain Bass): its compile() runs generate_event_semaphores,
    # which splits multi-wait sync conditions into event-semaphore chains —
    # TRN2 instructions hold at most one sync wait.
    nc = bacc.Bacc("TRN2", target_bir_lowering=False)
    xin = nc.dram_tensor("xin", [P, F_PER_CORE], mybir.dt.float32, kind="ExternalInput")
    yout = nc.dram_tensor("yout", [P, F_PER_CORE], mybir.dt.float32, kind="ExternalOutput")

    with tile.TileContext(nc) as tc:
        # bufs == n_chunks: every tile gets its own SBUF slot, so no
        # slot-reuse WAR waits — the TensorScalar ISA slot only fits 2 sync
        # waits and a third (prior out-DMA on a reused slot) overflows it.
        with tc.tile_pool(name="buf", bufs=len(CHUNKS)) as pool:
            off = 0
            for ch in CHUNKS:
                t = pool.tile([P, ch], mybir.dt.float32)
                nc.sync.dma_start(out=t[:, :], in_=xin[:, off : off + ch])
                nc.vector.tensor_scalar_add(t[:, :], t[:, :], float(c))
                nc.sync.dma_start(out=yout[:, off : off + ch], in_=t[:, :])
                off += ch
    nc.compile()
    return nc


def _run_const_add(x_flat: np.ndarray, c: float) -> np.ndarray:
    from concourse.bass_utils import run_bass_kernel_spmd

    key = ("const_add", float(c))
    nc = _PROG_CACHE.get(key)
    if nc is None:
        nc = _build_const_add(c)
        _PROG_CACHE[key] = nc

    per_core = x_flat.size // N_CORES
    shards = [
        np.ascontiguousarray(
            x_flat[k * per_core : (k + 1) * per_core].reshape(P, F_PER_CORE)
        )
        for k in range(N_CORES)
    ]
    res = run_bass_kernel_spmd(nc, [{"xin": s} for s in shards], list(range(N_CORES)))
    return np.concatenate([r["yout"].reshape(-1) for r in res.results])


# --------------------------------------------------------------------------
# Entry point
# --------------------------------------------------------------------------

def kernel(x, dw_k, dw_b, w0, b0, ws, bs, gamma, beta, mmean, mvar, wf, bf):
    x = np.ascontiguousarray(np.asarray(x, dtype=np.float32))
    args = (dw_k, dw_b, w0, b0, ws, bs, gamma, beta, mmean, mvar, wf, bf)
    args = tuple(np.asarray(a, dtype=np.float32) for a in args)
    (dw_k, dw_b, w0, b0, ws, bs, gamma, beta, mmean, mvar, wf, bf) = args

    K, zb, alphas, betas = _fold(*args)
    x_absmax = float(np.abs(x).max())
    collapse_at = _find_collapse(K, zb, alphas, betas, x_absmax)

    shardable = (x.size // N_CORES) == P * F_PER_CORE and x.size % N_CORES == 0
    if collapse_at is None or not shardable:
        return _host_reference(x, *args)

    c = _collapsed_const(collapse_at, ws, bs, gamma, beta, mmean, mvar, wf, bf)
    out_flat = _run_const_add(x.reshape(-1), float(c))
    return out_flat.reshape(x.shape).astype(np.float32)


# revision 9
# speedup vs baseline: 1.0452x; 1.0452x over previous
"""Trainium2 kernel for nn_CA_23175643529789 (dense_cnn, memory regime).

The reference network is:
    y  = depthwise3x3(x, dw_k, depth_multiplier=3) + dw_b      # 1 -> 3 ch
    h  = BN_0(relu(y @ w0 + b0))                               # 3 -> 1 ch
    h  = BN_{i+1}(relu(h * ws[i] + bs[i]))   for i in 0..9     # 1 -> 1 ch
    out = x + h * wf + bf

Everything after the depthwise conv is scalar arithmetic per pixel, so the
whole network folds (exactly, by linearity) into ONE 3x3 conv followed by a
chain of 11 scalar relu-affine stages:  v_{i+1} = alpha_i * relu(v_i) + beta_i,
with out = x + v_11.

At kernel-call time we know the actual weight values, so we propagate the
achievable value interval through the chain.  A stage whose input interval is
entirely <= 0 zeroes every pixel, making the rest of the chain a constant:
out = x + C.  (With the shipped weights this provably happens at stage 2 for
*any* input x, because alpha_1 < 0 and beta_1 < 0.)  The device kernel is then
a pure memory-roofline pass: read x, add C, write out, sharded over 8 cores.

If the collapse does not hold for the supplied weights, we fall back to an
exact host computation (correct, just not accelerated).
"""

import sys

import numpy as np

_REPO = "/opt/trn_rl_repo"
if _REPO not in sys.path:
    sys.path.insert(0, _REPO)

BN_EPS = 1e-3
N_CORES = 8

_PROG_CACHE: dict = {}


# --------------------------------------------------------------------------
# Host-side algebraic folding
# --------------------------------------------------------------------------

def _fold(dw_k, dw_b, w0, b0, ws, bs, gamma, beta, mmean, mvar, wf, bf):
    """Fold network into (K3x3, zbias, alphas[11], betas[11]) in float64."""
    f8 = np.float64
    K = np.einsum("dtj,j->dt", dw_k[:, :, 0, :].astype(f8), w0[:, 0].astype(f8))
    zb = float(np.dot(dw_b.astype(f8), w0[:, 0].astype(f8)) + f8(b0[0]))
    s = gamma[:, 0].astype(f8) / np.sqrt(mvar[:, 0].astype(f8) + BN_EPS)
    t = beta[:, 0].astype(f8) - mmean[:, 0].astype(f8) * s
    alphas, betas = [], []
    for i in range(10):
        alphas.append(float(s[i] * f8(ws[i, 0, 0])))
        betas.append(float(t[i] * f8(ws[i, 0, 0]) + f8(bs[i, 0])))
    alphas.append(float(s[10] * f8(wf[0, 0])))
    betas.append(float(t[10] * f8(wf[0, 0]) + f8(bf[0])))
    return K, zb, alphas, betas


def _find_collapse(K, zb, alphas, betas, x_absmax):
    """Interval-propagate; return stage index where relu provably zeroes
    every pixel (with margin), or None."""
    zr = float(np.abs(K).sum() * x_absmax)
    vlo, vhi = zb - zr, zb + zr
    for i in range(11):
        if vhi <= -1e-4:  # relu_i kills everything, with margin
            return i
        ulo, uhi = max(vlo, 0.0), max(vhi, 0.0)
        lo2 = alphas[i] * ulo + betas[i]
        hi2 = alphas[i] * uhi + betas[i]
        vlo, vhi = min(lo2, hi2), max(lo2, hi2)
    return None


def _collapsed_const(collapse_at, ws, bs, gamma, beta, mmean, mvar, wf, bf):
    """Replicate the reference's float32 arithmetic from block `collapse_at`
    (whose relu output is exactly 0 at every pixel) to the end."""
    f4 = np.float32
    gamma = gamma.astype(f4)
    beta = beta.astype(f4)
    mmean = mmean.astype(f4)
    mvar = mvar.astype(f4)
    ws = ws.astype(f4)
    bs = bs.astype(f4)

    def bn(u, k):
        return (u - mmean[k, 0]) * (gamma[k, 0] / np.sqrt(mvar[k, 0] + f4(BN_EPS))) + beta[k, 0]

    h = bn(f4(0.0), collapse_at)
    for k in range(collapse_at + 1, 11):
        h = bn(np.maximum(h * ws[k - 1, 0, 0] + bs[k - 1, 0], f4(0.0)), k)
    return f4(h * f4(wf[0, 0]) + f4(bf[0]))


# --------------------------------------------------------------------------
# Exact host fallback (only used if the collapse does not hold)
# --------------------------------------------------------------------------

def _host_reference(x, dw_k, dw_b, w0, b0, ws, bs, gamma, beta, mmean, mvar, wf, bf):
    f4 = np.float32
    B, H, W, C = x.shape
    xp = np.pad(x[..., 0], ((0, 0), (1, 1), (1, 1))).astype(f4)
    y = np.zeros((B, H, W, 3), dtype=f4)
    for j in range(3):
        acc = np.zeros((B, H, W), dtype=f4)
        for d in range(3):
            for tt in range(3):
                acc += dw_k[d, tt, 0, j] * xp[:, d : d + H, tt : tt + W]
        y[..., j] = acc + dw_b[j]

    def bn(u, k):
        return (u - mmean[k, 0]) * (gamma[k, 0] / np.sqrt(mvar[k, 0] + f4(BN_EPS))) + beta[k, 0]

    h = bn(np.maximum(y @ w0.astype(f4) + b0.astype(f4), 0.0)[..., 0], 0)
    for i in range(10):
        h = bn(np.maximum(h * ws[i, 0, 0] + bs[i, 0], 0.0), i + 1)
    dx = h * wf[0, 0] + bf[0]
    return (x + dx[..., None]).astype(f4)


# --------------------------------------------------------------------------
# Device program: out = x + C, sharded over 8 cores
# --------------------------------------------------------------------------

P = 128          # SBUF partitions
F_PER_CORE = 16384   # fp32 elems per partition per core (2*1024*1024 / 128)
CHUNK = 4096     # 4 chunks of 2 MiB


def _build_const_add(c: float):
    """Raw bass (no TileContext): the program is a 3-stage pipeline with
    exactly three semaphores, so we skip Tile's ~15 us of entry/exit
    barrier + event-semaphore overhead, and each engine issues its own
    stream independently:
      Sync   : all in-DMAs issued up front (pure prefetch, own HWDGE ring)
      Vector : in-place (x + c) per chunk as soon as its DMA lands
      Scalar : out-DMAs (separate HWDGE ring), final completion wait
    """
    import concourse.bass as bass
    from concourse import mybir

    n_chunks = F_PER_CORE // CHUNK
    nc = bass.Bass(target_bir_lowering=False)
    xin = nc.dram_tensor("xin", [P, F_PER_CORE], mybir.dt.float32, kind="ExternalInput")
    yout = nc.dram_tensor("yout", [P, F_PER_CORE], mybir.dt.float32, kind="ExternalOutput")
    bufs = [
        nc.alloc_sbuf_tensor(f"buf{k}", [P, CHUNK], mybir.dt.float32)
        for k in range(n_chunks)
    ]

    with (
        nc.Block() as block,
        nc.semaphore("in_sem") as in_sem,
        nc.semaphore("add_sem") as add_sem,
        nc.semaphore("out_sem") as out_sem,
    ):

        @block.sync
        def _(sync):
            for k in range(n_chunks):
                sync.dma_start(
                    out=bufs[k].ap()[:, :],
                    in_=xin[:, k * CHUNK : (k + 1) * CHUNK],
                ).then_inc(in_sem, 16)

        @block.vector
        def _(vector):
            for k in range(n_chunks):
                vector.wait_ge(in_sem, 16 * (k + 1))
                vector.tensor_scalar_add(
                    bufs[k].ap()[:, :], bufs[k].ap()[:, :], float(c)
                ).then_inc(add_sem, 1)

        @block.scalar
        def _(scalar):
            for k in range(n_chunks):
                scalar.wait_ge(add_sem, k + 1)
                scalar.dma_start(
                    out=yout[:, k * CHUNK : (k + 1) * CHUNK],
                    in_=bufs[k].ap()[:, :],
                ).then_inc(out_sem, 16)
            scalar.wait_ge(out_sem, 16 * n_chunks)

    return nc


def _run_const_add(x_flat: np.ndarray, c: float) -> np.ndarray:
    from concourse.bass_utils import run_bass_kernel_spmd

    key = ("const_add", float(c))
    nc = _PROG_CACHE.get(key)
    if nc is None:
        nc = _build_const_add(c)
        _PROG_CACHE[key] = nc

    per_core = x_flat.size // N_CORES
    shards = [
        np.ascontiguousarray(
            x_flat[k * per_core : (k + 1) * per_core].reshape(P, F_PER_CORE)
        )
        for k in range(N_CORES)
    ]
    res = run_bass_kernel_spmd(nc, [{"xin": s} for s in shards], list(range(N_CORES)))
    return np.concatenate([r["yout"].reshape(-1) for r in res.results])


# --------------------------------------------------------------------------
# Entry point
# --------------------------------------------------------------------------

def kernel(x, dw_k, dw_b, w0, b0, ws, bs, gamma, beta, mmean, mvar, wf, bf):
    x = np.ascontiguousarray(np.asarray(x, dtype=np.float32))
    args = (dw_k, dw_b, w0, b0, ws, bs, gamma, beta, mmean, mvar, wf, bf)
    args = tuple(np.asarray(a, dtype=np.float32) for a in args)
    (dw_k, dw_b, w0, b0, ws, bs, gamma, beta, mmean, mvar, wf, bf) = args

    K, zb, alphas, betas = _fold(*args)
    x_absmax = float(np.abs(x).max())
    collapse_at = _find_collapse(K, zb, alphas, betas, x_absmax)

    shardable = (x.size // N_CORES) == P * F_PER_CORE and x.size % N_CORES == 0
    if collapse_at is None or not shardable:
        return _host_reference(x, *args)

    c = _collapsed_const(collapse_at, ws, bs, gamma, beta, mmean, mvar, wf, bf)
    out_flat = _run_const_add(x.reshape(-1), float(c))
    return out_flat.reshape(x.shape).astype(np.float32)
